# revision 10
# baseline (speedup 1.0000x reference)
"""Trainium2 Bass kernel for nn_ExactPosteriorECD (2-layer GCN x2 towers + pair posterior fusion).

Strategy (8 NeuronCores, SPMD, 4 launches):
  L1: y^T = (x @ W1cat)^T            row-sharded GEMM, bf16, PE-bound
  L2: z^T = (A @ y)^T, h=relu, m = h @ W2blk   row-sharded, bf16
  L3: T = A @ [m_hi|m_lo] (+b2), softmax/sigmoid, scale -> fused node table
  L4: pair fusion over (u,v) batch: dma_gather table rows, DVE elementwise
Host does only data prep/resharding: graph normalization constants, padding,
transposes, dtype casts, and gather/unshard between launches.
"""

import numpy as np
import ml_dtypes
from contextlib import ExitStack

import concourse.bass as bass
import concourse.tile as tile
from concourse import bacc, mybir
from concourse import bass_utils

BF16 = ml_dtypes.bfloat16

# ---- problem shapes (hardcoded per contest contract) ----
N = 10000          # nodes (also feature dim)
C = 16             # communities
HID = 1024         # per-tower hidden
H2 = 2 * HID       # both towers stacked
B = 2000000        # query pairs
S_HP, H_HP = 2.0, 10.0
EPS = 1e-10

NCORES = 8
P = 128
NPAD = 10240       # padded node count (80*128); row 10000 is the bias node
RPC = NPAD // NCORES   # 1280 rows per core
KC = NPAD // P         # 80 contraction chunks
BPAD = 2000896         # padded pair count (8 * 250112)
BPC = BPAD // NCORES   # 250112 = 1954*128 pairs per core
TPC = BPC // P         # 1954 pair-tiles per core
CHG = 64               # gather-chunk size in pair-groups (64*128 = 8192 pairs)

_prog_cache = {}
LAUNCH_TIMES = []


def _f32(x):
    return np.ascontiguousarray(np.asarray(x), dtype=np.float32)


def _bf16(x):
    return np.ascontiguousarray(np.asarray(x, dtype=np.float32).astype(BF16))


# --------------------------------------------------------------------------
# launch builders
# --------------------------------------------------------------------------

def _build_l1(rep=1):
    """y^T[h, n] per core: lhsT/stationary = W1 m-chunks, moving = xT n-blocks.
    inputs: xT (10240,1280) bf16 per-core slice, W1 (10240,2048) bf16 replicated.
    output: yT (2048,1280) bf16."""
    nc = bacc.Bacc("TRN2", target_bir_lowering=False, debug=False,
                   num_devices=NCORES)
    xT = nc.dram_tensor("xT", [NPAD, RPC], mybir.dt.bfloat16,
                        kind="ExternalInput").ap()
    w1 = nc.dram_tensor("w1", [NPAD, H2], mybir.dt.bfloat16,
                        kind="ExternalInput").ap()
    yT = nc.dram_tensor("yT", [H2, RPC], mybir.dt.bfloat16,
                        kind="ExternalOutput").ap()

    nblocks = [(0, 512), (512, 512), (1024, 256)]   # over RPC=1280
    with tile.TileContext(nc) as tc, ExitStack() as ctx:
        mov = ctx.enter_context(tc.tile_pool(name="mov", bufs=1))
        sta = ctx.enter_context(tc.tile_pool(name="sta", bufs=2))
        out = ctx.enter_context(tc.tile_pool(name="out", bufs=3))
        ps = ctx.enter_context(tc.tile_pool(name="ps", bufs=2, space="PSUM"))
        for n0, nsz in nblocks * rep:
            xt_t = mov.tile([P, KC, nsz], mybir.dt.bfloat16, tag="mov")
            nc.sync.dma_start(
                xt_t[:], xT[:, n0:n0 + nsz].rearrange("(kc p) n -> p kc n", p=P))
            for mb2 in range(H2 // 256):     # stationary in 256-col slabs
                w_t = sta.tile([P, KC, 256], mybir.dt.bfloat16, tag="sta")
                nc.sync.dma_start(
                    w_t[:],
                    w1[:, mb2 * 256:(mb2 + 1) * 256].rearrange(
                        "(kc p) m -> p kc m", p=P))
                for mh in range(2):          # two 128-wide m-tiles per slab
                    acc = ps.tile([P, nsz], mybir.dt.float32, tag="ps")
                    for kc in range(KC):
                        nc.tensor.matmul(
                            acc[:], lhsT=w_t[:, kc, mh * P:(mh + 1) * P],
                            rhs=xt_t[:, kc, :],
                            start=(kc == 0), stop=(kc == KC - 1))
                    ot = out.tile([P, nsz], mybir.dt.bfloat16, tag="out")
                    nc.vector.tensor_copy(ot[:], acc[:])
                    m0 = mb2 * 256 + mh * P
                    nc.sync.dma_start(yT[m0:m0 + P, n0:n0 + nsz], ot[:])
    nc.compile()
    return nc


def _build_l2(rep=1):
    """z^T = (A_k @ y)^T + b1 (bias via node-10000 trick), h=relu(z) (bf16,
    SBUF-resident), m = h @ W2blk.
    inputs: y (10240,2048) bf16 replicated, aT (10240,1280) bf16 per-core,
            w2 (2048,64) bf16 blockdiag replicated.
    output: m (1280,32) f32."""
    nc = bacc.Bacc("TRN2", target_bir_lowering=False, debug=False,
                   num_devices=NCORES)
    y = nc.dram_tensor("y", [NPAD, H2], mybir.dt.bfloat16,
                       kind="ExternalInput").ap()
    aT = nc.dram_tensor("aT", [NPAD, RPC], mybir.dt.bfloat16,
                        kind="ExternalInput").ap()
    w2 = nc.dram_tensor("w2", [H2, 32], mybir.dt.bfloat16,
                        kind="ExternalInput").ap()
    m_out = nc.dram_tensor("m", [RPC, 32], mybir.dt.float32,
                           kind="ExternalOutput").ap()

    nblocks = [(0, 512), (512, 512), (1024, 256)]
    HKC = H2 // P   # 16 contraction chunks for GEMM2
    with tile.TileContext(nc) as tc, ExitStack() as ctx:
        mov = ctx.enter_context(tc.tile_pool(name="mov", bufs=1))
        sta = ctx.enter_context(tc.tile_pool(name="sta", bufs=2))
        hres = ctx.enter_context(tc.tile_pool(name="hres", bufs=1))
        wp = ctx.enter_context(tc.tile_pool(name="wp", bufs=1))
        out = ctx.enter_context(tc.tile_pool(name="out", bufs=2))
        ps = ctx.enter_context(tc.tile_pool(name="ps", bufs=2, space="PSUM"))
        ps2 = ctx.enter_context(tc.tile_pool(name="ps2", bufs=2, space="PSUM"))

        # hT resident: [2048 hid, 1280 node] bf16 as [128, 16, 1280]
        hT = hres.tile([P, HKC, RPC], mybir.dt.bfloat16)
        w2_t = wp.tile([P, HKC, 32], mybir.dt.bfloat16)
        nc.sync.dma_start(w2_t[:], w2.rearrange("(kc p) c -> p kc c", p=P))

        for n0, nsz in nblocks * rep:
            at_t = mov.tile([P, KC, nsz], mybir.dt.bfloat16, tag="mov")
            nc.sync.dma_start(
                at_t[:], aT[:, n0:n0 + nsz].rearrange("(kc p) n -> p kc n", p=P))
            for mb2 in range(H2 // 256):
                y_t = sta.tile([P, KC, 256], mybir.dt.bfloat16, tag="sta")
                nc.sync.dma_start(
                    y_t[:],
                    y[:, mb2 * 256:(mb2 + 1) * 256].rearrange(
                        "(kc p) m -> p kc m", p=P))
                for mh in range(2):
                    acc = ps.tile([P, nsz], mybir.dt.float32, tag="ps")
                    for kc in range(KC):
                        nc.tensor.matmul(
                            acc[:], lhsT=y_t[:, kc, mh * P:(mh + 1) * P],
                            rhs=at_t[:, kc, :],
                            start=(kc == 0), stop=(kc == KC - 1))
                    # relu + cast into resident hT
                    mi = mb2 * 2 + mh      # hid-chunk index 0..15
                    nc.vector.tensor_scalar_max(
                        hT[:, mi, n0:n0 + nsz], acc[:], 0.0)

        # GEMM2: m[node, 32] = h @ w2blk ; lhsT = hT chunks
        for mt in list(range(RPC // P)) * rep:
            acc2 = ps2.tile([P, 32], mybir.dt.float32, tag="ps2")
            for kc in range(HKC):
                nc.tensor.matmul(
                    acc2[:], lhsT=hT[:, kc, mt * P:(mt + 1) * P],
                    rhs=w2_t[:, kc, :],
                    start=(kc == 0), stop=(kc == HKC - 1))
            ot = out.tile([P, 32], mybir.dt.float32, tag="out")
            nc.vector.tensor_copy(ot[:], acc2[:])
            nc.sync.dma_start(m_out[mt * P:(mt + 1) * P, :], ot[:])
    nc.compile()
    return nc


def _build_l3(rep=1):
    """T_k = A_k @ [m_hi|m_lo] (b2 via bias node), softmax/sigmoid, scale.
    inputs: aT (10240,1280) bf16 per-core, mcat (10240,64) bf16 replicated,
            sc (128,64) f32 scale tile (sqrt(a),sqrt(1-a) replicated rows).
    output: tbl (1280,64) f32 = [theta*sa | phi*sp | 0 | 0]."""
    nc = bacc.Bacc("TRN2", target_bir_lowering=False, debug=False,
                   num_devices=NCORES)
    aT = nc.dram_tensor("aT", [NPAD, RPC], mybir.dt.bfloat16,
                        kind="ExternalInput").ap()
    mcat = nc.dram_tensor("mcat", [NPAD, 64], mybir.dt.bfloat16,
                          kind="ExternalInput").ap()
    sc = nc.dram_tensor("sc", [P, 64], mybir.dt.float32,
                        kind="ExternalInput").ap()
    tbl = nc.dram_tensor("tbl", [RPC, 64], mybir.dt.float32,
                         kind="ExternalOutput").ap()

    with tile.TileContext(nc) as tc, ExitStack() as ctx:
        sta = ctx.enter_context(tc.tile_pool(name="sta", bufs=2))
        mv = ctx.enter_context(tc.tile_pool(name="mv", bufs=1))
        scp = ctx.enter_context(tc.tile_pool(name="scp", bufs=1))
        wk = ctx.enter_context(tc.tile_pool(name="wk", bufs=2))
        ps = ctx.enter_context(tc.tile_pool(name="ps", bufs=2, space="PSUM"))

        m_t = mv.tile([P, KC, 64], mybir.dt.bfloat16)
        nc.sync.dma_start(m_t[:], mcat.rearrange("(kc p) c -> p kc c", p=P))
        sc_t = scp.tile([P, 64], mybir.dt.float32)
        nc.sync.dma_start(sc_t[:], sc[:, :])

        for mb2 in list(range(RPC // 256)) * rep:
            a_t = sta.tile([P, KC, 256], mybir.dt.bfloat16, tag="sta")
            nc.sync.dma_start(
                a_t[:],
                aT[:, mb2 * 256:(mb2 + 1) * 256].rearrange(
                    "(kc p) m -> p kc m", p=P))
            for mh in range(2):
                acc = ps.tile([P, 64], mybir.dt.float32, tag="ps")
                for kc in range(KC):
                    nc.tensor.matmul(
                        acc[:], lhsT=a_t[:, kc, mh * P:(mh + 1) * P],
                        rhs=m_t[:, kc, :],
                        start=(kc == 0), stop=(kc == KC - 1))
                # T = hi + lo halves (PSUM allows only one PSUM operand)
                acc_sb = wk.tile([P, 64], mybir.dt.float32, tag="accsb")
                nc.vector.tensor_copy(acc_sb[:], acc[:])
                t_t = wk.tile([P, 32], mybir.dt.float32, tag="T")
                nc.vector.tensor_tensor(t_t[:], acc_sb[:, 0:32], acc_sb[:, 32:64],
                                        op=mybir.AluOpType.add)
                # softmax over first 16 cols
                negmax = wk.tile([P, 1], mybir.dt.float32, tag="negmax")
                nc.vector.tensor_reduce(negmax[:], t_t[:, 0:C],
                                        axis=mybir.AxisListType.X,
                                        op=mybir.AluOpType.max, negate=True)
                expt = wk.tile([P, C], mybir.dt.float32, tag="expt")
                sumexp = wk.tile([P, 1], mybir.dt.float32, tag="sumexp")
                nc.scalar.activation(expt[:], t_t[:, 0:C],
                                     mybir.ActivationFunctionType.Exp,
                                     bias=negmax[:, 0:1], scale=1.0,
                                     accum_out=sumexp[:, 0:1])
                rinv = wk.tile([P, 1], mybir.dt.float32, tag="rinv")
                nc.vector.reciprocal(rinv[:], sumexp[:])
                t1 = wk.tile([P, 1], mybir.dt.float32, tag="t1")
                nc.vector.tensor_tensor(t1[:], sumexp[:], rinv[:],
                                        op=mybir.AluOpType.mult)
                nc.vector.tensor_scalar(t1[:], t1[:], -1.0, 2.0,
                                        op0=mybir.AluOpType.mult,
                                        op1=mybir.AluOpType.add)
                nc.vector.tensor_tensor(rinv[:], rinv[:], t1[:],
                                        op=mybir.AluOpType.mult)
                # sigmoid over cols 16:32
                sig = wk.tile([P, C], mybir.dt.float32, tag="sig")
                nc.scalar.activation(sig[:], t_t[:, C:32],
                                     mybir.ActivationFunctionType.Sigmoid)
                ot = wk.tile([P, 64], mybir.dt.float32, tag="ot")
                nc.gpsimd.memset(ot[:], 0.0)
                # theta * sa  (theta = expt * rinv, folded via tensor_scalar)
                nc.vector.tensor_scalar(ot[:, 0:C], expt[:], rinv[:, 0:1],
                                        None, op0=mybir.AluOpType.mult)
                nc.vector.tensor_tensor(ot[:, 0:C], ot[:, 0:C], sc_t[:, 0:C],
                                        op=mybir.AluOpType.mult)
                nc.vector.tensor_tensor(ot[:, C:32], sig[:], sc_t[:, C:32],
                                        op=mybir.AluOpType.mult)
                mt0 = (mb2 * 2 + mh) * P
                nc.sync.dma_start(tbl[mt0:mt0 + P, :], ot[:])
    nc.compile()
    return nc


def _build_l4(rep=1):
    """Pair fusion. inputs: tbl (10000,64) f32, uw/vw (128, TPC*8) int16
    wrapped indices, prb (128,CHG,16) f32 prior bcast tile.
    outputs: p,q (128, TPC, 16) f32 partition-major (host unshuffles)."""
    nc = bacc.Bacc("TRN2", target_bir_lowering=False, debug=False,
                   num_devices=NCORES, num_swdge_queues=4)
    tbl = nc.dram_tensor("tbl", [N, 64], mybir.dt.float32,
                         kind="ExternalInput").ap()
    IW = BPC // 16        # wrapped idx columns = 15632
    uw = nc.dram_tensor("uw", [P, IW], mybir.dt.int16,
                        kind="ExternalInput").ap()
    vw = nc.dram_tensor("vw", [P, IW], mybir.dt.int16,
                        kind="ExternalInput").ap()
    prb = nc.dram_tensor("prb", [P, CHG, C], mybir.dt.float32,
                         kind="ExternalInput").ap()
    p_out = nc.dram_tensor("p", [P, TPC, C], mybir.dt.float32,
                           kind="ExternalOutput").ap()
    q_out = nc.dram_tensor("q", [P, TPC, C], mybir.dt.float32,
                           kind="ExternalOutput").ap()

    chunks = []
    g0 = 0
    while g0 < TPC:
        g = min(CHG, TPC - g0)
        chunks.append((g0, g))
        g0 += g

    with tile.TileContext(nc) as tc, ExitStack() as ctx:
        idxp = ctx.enter_context(tc.tile_pool(name="idxp", bufs=1))
        cst = ctx.enter_context(tc.tile_pool(name="cst", bufs=1))
        gat = ctx.enter_context(tc.tile_pool(name="gat", bufs=2))
        wk = ctx.enter_context(tc.tile_pool(name="wk", bufs=2))

        uw_t = idxp.tile([P, IW], mybir.dt.int16, tag="uw")
        nc.sync.dma_start(uw_t[:], uw[:, :])
        vw_t = idxp.tile([P, IW], mybir.dt.int16, tag="vw")
        nc.sync.dma_start(vw_t[:], vw[:, :])
        prb_t = cst.tile([P, CHG, C], mybir.dt.float32)
        nc.sync.dma_start(prb_t[:], prb[:, :, :])

        for ci, (g0, g) in enumerate(chunks * rep):
            qn = ci % 4
            gu = gat.tile([P, CHG, 64], mybir.dt.float32, tag="gu")
            nc.gpsimd.dma_gather(
                gu[:, 0:g, :], tbl[:, :], uw_t[:, g0 * 8:(g0 + g) * 8],
                num_idxs=g * P, num_idxs_reg=g * P, elem_size=64,
                queue_num=qn, single_packet=False)
            gv = gat.tile([P, CHG, 64], mybir.dt.float32, tag="gv")
            nc.gpsimd.dma_gather(
                gv[:, 0:g, :], tbl[:, :], vw_t[:, g0 * 8:(g0 + g) * 8],
                num_idxs=g * P, num_idxs_reg=g * P, elem_size=64,
                queue_num=qn, single_packet=False)
            prod = wk.tile([P, CHG, 32], mybir.dt.float32, tag="prod")
            nc.vector.tensor_tensor(prod[:, 0:g, :], gu[:, 0:g, 0:32],
                                    gv[:, 0:g, 0:32],
                                    op=mybir.AluOpType.mult)
            pt = wk.tile([P, CHG, C], mybir.dt.float32, tag="pt")
            nc.vector.tensor_tensor(pt[:, 0:g, :], prod[:, 0:g, 0:C],
                                    prod[:, 0:g, C:32],
                                    op=mybir.AluOpType.add)
            nc.vector.tensor_scalar_add(pt[:, 0:g, :], pt[:, 0:g, :], EPS)
            qt = wk.tile([P, CHG, C], mybir.dt.float32, tag="qt")
            nc.vector.tensor_tensor(qt[:, 0:g, :], pt[:, 0:g, :],
                                    prb_t[:, 0:g, :],
                                    op=mybir.AluOpType.mult)
            ssum = wk.tile([P, CHG], mybir.dt.float32, tag="ssum")
            nc.vector.tensor_reduce(ssum[:, 0:g], qt[:, 0:g, :],
                                    axis=mybir.AxisListType.X,
                                    op=mybir.AluOpType.add)
            rinv = wk.tile([P, CHG], mybir.dt.float32, tag="rinv")
            nc.vector.reciprocal(rinv[:, 0:g], ssum[:, 0:g])
            # one Newton step: r = r*(2 - s*r) (hw reciprocal is approximate)
            t1 = wk.tile([P, CHG], mybir.dt.float32, tag="t1")
            nc.vector.tensor_tensor(t1[:, 0:g], ssum[:, 0:g], rinv[:, 0:g],
                                    op=mybir.AluOpType.mult)
            nc.vector.tensor_scalar(t1[:, 0:g], t1[:, 0:g], -1.0, 2.0,
                                    op0=mybir.AluOpType.mult,
                                    op1=mybir.AluOpType.add)
            nc.vector.tensor_tensor(rinv[:, 0:g], rinv[:, 0:g], t1[:, 0:g],
                                    op=mybir.AluOpType.mult)
            nc.vector.tensor_tensor(
                qt[:, 0:g, :], qt[:, 0:g, :],
                rinv[:, 0:g].unsqueeze(2).to_broadcast([P, g, C]),
                op=mybir.AluOpType.mult)
            nc.sync.dma_start(p_out[:, g0:g0 + g, :], pt[:, 0:g, :])
            nc.sync.dma_start(q_out[:, g0:g0 + g, :], qt[:, 0:g, :])
    nc.compile()
    return nc


BUILDERS = {"l1": _build_l1, "l2": _build_l2, "l3": _build_l3, "l4": _build_l4}


def _get_prog(name, rep=1):
    key = (name, rep)
    if key not in _prog_cache:
        _prog_cache[key] = BUILDERS[name](rep=rep)
    return _prog_cache[key]


def _run(nc, in_maps, name=""):
    import time as _time
    t0 = _time.perf_counter()
    res = bass_utils.run_bass_kernel_spmd(
        nc, in_maps, core_ids=list(range(NCORES)))
    LAUNCH_TIMES.append((name, _time.perf_counter() - t0))
    return res.results


# --------------------------------------------------------------------------
# host orchestration
# --------------------------------------------------------------------------

def kernel(node_features, edge_index, u, v, tW1, tb1, tW2, tb2,
           pW1, pb1, pW2, pb2, eta_param):
    x = _f32(node_features)
    edge_index = np.asarray(edge_index)
    u = np.asarray(u)
    v = np.asarray(v)
    tb1 = _f32(tb1); pb1 = _f32(pb1); tb2 = _f32(tb2); pb2 = _f32(pb2)
    eta_param = _f32(eta_param)

    # ---- graph normalization (host: O(E) index prep) ----
    src = np.concatenate([np.asarray(edge_index[0], np.int64), np.arange(N)])
    dst = np.concatenate([np.asarray(edge_index[1], np.int64), np.arange(N)])
    deg = np.bincount(dst, minlength=N).astype(np.float32)
    dinv = np.where(deg > 0, deg ** -0.5, 0.0).astype(np.float32)
    norm = dinv[src] * dinv[dst]

    A = np.zeros((NPAD, NPAD), np.float32)
    np.add.at(A, (dst, src), norm)
    A[:N, N] = 1.0                      # bias-node column
    AT16 = np.ascontiguousarray(A.T).astype(BF16)   # [NPAD src, NPAD dst]
    del A

    # ---- L1: y = x @ W1cat ----
    xT = np.zeros((NPAD, NPAD), BF16)
    xT[:N, :N] = x.T.astype(BF16)
    W1 = np.zeros((NPAD, H2), BF16)
    W1[:N, :HID] = _bf16(tW1)
    W1[:N, HID:] = _bf16(pW1)

    l1 = _get_prog("l1")
    in_maps = [{"xT": np.ascontiguousarray(xT[:, k * RPC:(k + 1) * RPC]),
                "w1": W1} for k in range(NCORES)]
    r1 = _run(l1, in_maps, "l1")
    yT = np.concatenate([r1[k]["yT"] for k in range(NCORES)], axis=1)  # [H2, NPAD]
    del in_maps

    y = np.zeros((NPAD, H2), BF16)
    y[:, :] = np.ascontiguousarray(yT.T)
    # bias node row: z = A@y + b1 via A[:,bias]=1, y[bias]=b1
    y[N, :HID] = tb1.astype(BF16)
    y[N, HID:] = pb1.astype(BF16)
    y[N + 1:, :] = 0
    del yT

    # ---- L2: z = A@y + b1, h = relu(z), m = h @ W2blk ----
    W2blk = np.zeros((H2, 32), BF16)
    W2blk[:HID, :C] = _bf16(tW2)
    W2blk[HID:, C:] = _bf16(pW2)
    l2 = _get_prog("l2")
    in_maps = [{"y": y,
                "aT": np.ascontiguousarray(AT16[:, k * RPC:(k + 1) * RPC]),
                "w2": W2blk} for k in range(NCORES)]
    r2 = _run(l2, in_maps, "l2")
    m = np.concatenate([r2[k]["m"] for k in range(NCORES)], axis=0)  # [NPAD,32] f32
    del in_maps

    # bias node row for layer-2 agg; hi/lo split for ~fp32 logits
    m[N, :C] = tb2
    m[N, C:] = pb2
    m[N + 1:, :] = 0
    m_hi = m.astype(BF16)
    m_lo = (m - m_hi.astype(np.float32)).astype(BF16)
    mcat = np.concatenate([m_hi, m_lo], axis=1)      # [NPAD, 64] bf16

    # ---- fusion constants from eta ----
    eta = np.tanh(eta_param).astype(np.float32)
    a = np.abs(eta)
    alpha = H_HP * a + S_HP * (1.0 - a)
    prior = (alpha / alpha.sum()).astype(np.float32)
    sc = np.zeros((P, 64), np.float32)
    sc[:, 0:C] = np.sqrt(a)
    sc[:, C:32] = np.sqrt(1.0 - a)

    # ---- L3: node tables ----
    l3 = _get_prog("l3")
    in_maps = [{"aT": np.ascontiguousarray(AT16[:, k * RPC:(k + 1) * RPC]),
                "mcat": mcat, "sc": sc} for k in range(NCORES)]
    r3 = _run(l3, in_maps, "l3")
    tbl = np.concatenate([r3[k]["tbl"] for k in range(NCORES)], axis=0)[:N]
    tbl = np.ascontiguousarray(tbl)                  # [N, 64] f32
    del in_maps

    # ---- L4: pair fusion ----
    upad = np.zeros(BPAD, np.int16)
    vpad = np.zeros(BPAD, np.int16)
    upad[:B] = u.astype(np.int16)
    vpad[:B] = v.astype(np.int16)

    def wrap(idx_shard):
        w = idx_shard.reshape(-1, 16).T              # [16, BPC//16]
        return np.ascontiguousarray(np.tile(w, (8, 1)))

    prb = np.broadcast_to(prior, (P, CHG, C)).astype(np.float32)
    prb = np.ascontiguousarray(prb)

    l4 = _get_prog("l4")
    in_maps = [{"tbl": tbl,
                "uw": wrap(upad[k * BPC:(k + 1) * BPC]),
                "vw": wrap(vpad[k * BPC:(k + 1) * BPC]),
                "prb": prb} for k in range(NCORES)]
    r4 = _run(l4, in_maps, "l4")

    p_full = np.empty((BPAD, C), np.float32)
    q_full = np.empty((BPAD, C), np.float32)
    for k in range(NCORES):
        p_full[k * BPC:(k + 1) * BPC] = (
            r4[k]["p"].transpose(1, 0, 2).reshape(BPC, C))
        q_full[k * BPC:(k + 1) * BPC] = (
            r4[k]["q"].transpose(1, 0, 2).reshape(BPC, C))

    return q_full[:B], p_full[:B], eta
